# revision 11
# baseline (speedup 1.0000x reference)
import sys as _sys
for _p in ("/opt/trn_rl_repo", "/opt/pypackages"):
    if _p not in _sys.path:
        _sys.path.insert(0, _p)
"""GATv2 message-passing kernel for TRN2 (Bass/Tile), data-parallel over dst nodes.

V3 design:
  - Phase 1 builds a per-node table xl_tab[n] = 16*(x@Wl') (bf16, 656B-aligned
    rows) in device DRAM: 157 deep-pipelined matmul+drain+DMA iterations.
  - Phase 2 (per 128-edge chunk) gathers xl_tab[src] rows directly -- no
    per-edge transpose or xl recompute.  The gathered rows serve BOTH the
    value path (v = exp * xl) and the attention path, where they are injected
    into the m PSUM by an exact identity matmul (bf16).
  - Attention-only terms ea@We + MT@xr run as ONE fp8e4 DoubleRow matmul pair
    (K=256 at 0.5 cyc/col).  The x16 operand scale (fp8 subnormal avoidance)
    is undone for free via the relu activation scale and the denominator
    constant.  fp8 perturbs only attention logits (softmax re-normalizes).
  - Value scatter in bf16 (f32r at <256 cols is 4x slower on the PE).
  - Host degree-balances nodes across (core,group,slot) bins so all per-group
    edge counts are ~equal (TOTCH 200 vs 225 naive).
  - Chunk tables (combined fp8 lhsT, scatter one-hot) are DMA'd two chunks at
    a time: 512B/partition lines avoid the <512B half-bandwidth DMA penalty.
"""

import math
from contextlib import ExitStack
from dataclasses import dataclass, field

import numpy as np
import ml_dtypes

import concourse.bacc as bacc
import concourse.tile as tile
from concourse import bass, mybir
from concourse.masks import make_identity

F32 = mybir.dt.float32
BF16 = mybir.dt.bfloat16
FP8 = mybir.dt.float8e4
I32 = mybir.dt.int32

BN_EPS = 1e-5
NEG_SLOPE = 0.2
PAD_SENTINEL = 200.0  # pool one-hot compare value that never matches (> G)
WSCALE = 16.0         # fp8 range scale for attention-path operands


@dataclass
class Cfg:
    N: int
    E: int
    G: int
    n_cores: int
    F: int = 128
    H: int = 10
    C: int = 64
    GPC: int = 20     # 128-node groups per core
    Kg: list = field(default_factory=list)  # chunks per group (shared across cores)
    debug: bool = False

    @property
    def HC(self):
        return self.H * self.C

    @property
    def TOTCH(self):
        return sum(self.Kg)

    @property
    def NT(self):
        return ((self.N + 127) // 128) * 128


def fold_bn(inp):
    """Fold BatchNorm into the linear weights. Returns fp64 arrays."""
    g = np.float64(inp["bn_weight"]) / np.sqrt(np.float64(inp["bn_var"]) + BN_EPS)
    c0 = np.float64(inp["bn_bias"]) - np.float64(inp["bn_mean"]) * g
    Wl = g[:, None] * np.float64(inp["W_l"])
    Wr = g[:, None] * np.float64(inp["W_r"])
    bl = np.float64(inp["b_l"]) + c0 @ np.float64(inp["W_l"])
    br = np.float64(inp["b_r"]) + c0 @ np.float64(inp["W_r"])
    return Wl, Wr, bl + br, bl


def assign_nodes(dst, N, n_cores, GPC):
    """Degree-balanced assignment of nodes to (core, group, slot) bins."""
    import heapq
    deg = np.bincount(dst, minlength=N)
    nbins = n_cores * GPC
    cap = 128
    order = np.argsort(-deg, kind="stable")
    heap = [(0, b) for b in range(nbins)]
    heapq.heapify(heap)
    fill = np.zeros(nbins, np.int64)
    slot_of = np.empty(N, np.int64)
    spill = []
    for n in order:
        while True:
            load, b = heapq.heappop(heap)
            if fill[b] < cap:
                break
            spill.append((load, b))
        slot_of[n] = b * cap + fill[b]
        fill[b] += 1
        heapq.heappush(heap, (load + int(deg[n]), b))
        for it in spill:
            heapq.heappush(heap, it)
        spill.clear()
    counts = np.zeros((n_cores, GPC), np.int64)
    np.add.at(counts.reshape(-1), slot_of[dst] // cap, 1)
    return slot_of, counts


def preprocess(inp, n_cores, G):
    """Host-side sharding. Returns (cfg, in_maps, b_lin)."""
    x = np.asarray(inp["x"], np.float32)
    ea = np.asarray(inp["edge_attr"], np.float32)
    edge_index = np.asarray(inp["edge_index"], np.int64)
    batch = np.asarray(inp["batch"], np.int64)
    N, F = x.shape
    E = edge_index.shape[1]

    cfg = Cfg(N=N, E=E, G=G, n_cores=n_cores, F=F)
    GPC = cfg.GPC
    assert n_cores * GPC * 128 >= N

    Wl, Wr, bsum, bl_eff = fold_bn(inp)
    att = np.float64(np.asarray(inp["att"], np.float32)).reshape(-1)  # [H*C]
    We = np.float64(np.asarray(inp["W_e"], np.float32))
    bias = np.asarray(inp["bias"], np.float32)
    W_lin = np.asarray(inp["W_lin"], np.float32)
    b_lin = np.asarray(inp["b_lin"], np.float32)
    H, C, HC = cfg.H, cfg.C, cfg.HC
    HCH = HC + H

    src = edge_index[0].astype(np.int64)
    dst = edge_index[1].astype(np.int64)

    # --- balanced node -> (core, group, slot) assignment
    slot_of, counts = assign_nodes(dst, N, n_cores, GPC)
    Kg = np.maximum(1, np.ceil(counts / 128.0).astype(np.int64).max(axis=0))
    if Kg.sum() % 2 == 1:
        Kg[-1] += 1
    cfg.Kg = [int(k) for k in Kg]
    TOTCH = cfg.TOTCH
    PAIRS = TOTCH // 2
    chunk_base = np.concatenate([[0], np.cumsum(Kg)])

    # --- shared weight tables (att-projection trick: lrelu(m) = slope*m +
    # (1-slope)*relu(m); att.(slope*m) is linear in m so the extra H columns of
    # each weight block compute it inside the same matmuls).
    attm = att.reshape(H, C)

    def pad_att(W):
        Wp = np.zeros((W.shape[0], HCH), np.float64)
        Wp[:, :HC] = W
        for h in range(H):
            Wp[:, HC + h] = NEG_SLOPE * (W[:, h * C:(h + 1) * C] @ attm[h])
        return Wp

    wl_b = (WSCALE * pad_att(Wl)).astype(ml_dtypes.bfloat16)        # bf16, x16
    wr_b = (WSCALE * pad_att(Wr)).astype(ml_dtypes.bfloat16)        # bf16, x16
    we_8 = (WSCALE * pad_att(We)).astype(ml_dtypes.float8_e4m3)     # fp8, x16
    attb = np.broadcast_to(((1.0 - NEG_SLOPE) * att).astype(ml_dtypes.bfloat16),
                           (128, HC)).copy()
    bsum_att = np.concatenate([bsum, NEG_SLOPE * (bsum.reshape(H, C) * attm).sum(axis=1)])
    bsumb = np.broadcast_to((WSCALE * bsum_att).astype(np.float32), (128, HCH)).copy()
    # value-path b_l enters after softmax (weights sum to 1): fold its head-mean
    # into the output bias (exact for nodes with >=1 in-edge)
    bias_eff = bias + bl_eff.reshape(H, C).mean(axis=0).astype(np.float32)
    biasb = np.broadcast_to(bias_eff, (128, C)).copy().astype(np.float32)

    cnt = np.bincount(batch, minlength=G).astype(np.float32)
    cinv = (1.0 / np.maximum(cnt, 1.0)).reshape(G, 1).astype(np.float32)

    x_bf = x.astype(ml_dtypes.bfloat16)
    ea_bf = ea.astype(ml_dtypes.bfloat16)
    # x transposed in GLOBAL node order for the phase-1 table build
    xT = np.zeros((F, cfg.NT), ml_dtypes.bfloat16)
    xT[:, :N] = x_bf.T

    # --- per-core tables
    NS = GPC * 128  # slots per core
    core_of_e = slot_of[dst] // NS
    grp_of_e = (slot_of[dst] % NS) // 128
    order_e = np.lexsort((np.arange(E), slot_of[dst]))

    in_maps = []
    for c in range(n_cores):
        sel = core_of_e[order_e] == c
        eids_c = order_e[sel]
        grp_c = grp_of_e[order_e[sel]]
        slot = np.full(TOTCH * 128, -1, np.int64)
        for g in range(GPC):
            ge = eids_c[grp_c == g]
            base = chunk_base[g] * 128
            slot[base:base + len(ge)] = ge
        pad = slot < 0
        eidx = np.where(pad, 0, slot)

        srci = src[eidx].astype(np.int32)
        srci[pad] = 0
        srci = srci.reshape(TOTCH, 128).T.copy()  # [128, TOTCH]

        gidx = np.repeat(np.arange(TOTCH), 128)
        g_of_chunk = np.searchsorted(chunk_base[1:], gidx, side="right")
        dstl = (slot_of[dst[eidx]] % NS - g_of_chunk * 128).astype(np.int64)
        dstl[pad] = 10**6
        dstl2 = dstl.reshape(TOTCH, 128)
        onehot = dstl2[:, :, None] == np.arange(128)[None, None, :]  # [T, e, n]

        # scatter lhsT M_f[e, n], bf16, two chunks per 256-col row block
        m_f = onehot.astype(ml_dtypes.bfloat16)
        mf_pair = m_f.reshape(PAIRS, 2, 128, 128).transpose(0, 2, 1, 3) \
                     .reshape(PAIRS * 128, 256).copy()

        # combined DoubleRow lhsT [k=128, 2, e]: k-tile0 = eaT (fp8 data),
        # k-tile1 = MT one-hot; two chunks per 512B row block
        eat = ea_bf[eidx].astype(ml_dtypes.float8_e4m3)   # [T*128, F]
        eat[pad] = 0
        eatT = eat.reshape(TOTCH, 128, F).transpose(0, 2, 1)            # [T, F, e]
        mt_8 = onehot.transpose(0, 2, 1).astype(ml_dtypes.float8_e4m3)  # [T, n, e]
        lhst8 = np.empty((PAIRS, 128, 2, 2, 128), ml_dtypes.float8_e4m3)
        lhst8[:, :, 0, 0] = eatT[0::2]
        lhst8[:, :, 0, 1] = eatT[1::2]
        lhst8[:, :, 1, 0] = mt_8[0::2]
        lhst8[:, :, 1, 1] = mt_8[1::2]
        lhst8 = lhst8.reshape(PAIRS * 128, 2, 256).copy()

        # own nodes by slot, pre-transposed for the xr matmul lhsT
        own = (slot_of // NS) == c
        inv_idx = slot_of[own] % NS
        xo = np.zeros((NS, F), ml_dtypes.bfloat16)
        xo[inv_idx] = x_bf[own]
        xoT = xo.T.copy()  # [F, NS]

        bl = np.full(NS, int(PAD_SENTINEL), np.int64)
        bl[inv_idx] = batch[own]
        bloc = bl.reshape(GPC, 128).T.copy().astype(np.float32)  # [128, GPC]

        in_maps.append({
            "xT": xT, "xoT": xoT, "lhst8": lhst8, "mf": mf_pair,
            "srci": srci, "bloc": bloc,
            "wl": wl_b, "wr": wr_b, "we8": we_8,
            "attb": attb, "bsumb": bsumb, "biasb": biasb,
            "wlin": W_lin, "cinv": cinv,
        })
    return cfg, in_maps, b_lin


def build_kernel(cfg: Cfg):
    H, C, HC, F, G = cfg.H, cfg.C, cfg.HC, cfg.F, cfg.G
    GPC, Kg, TOTCH, NT = cfg.GPC, cfg.Kg, cfg.TOTCH, cfg.NT
    HCH = HC + H
    ROWW = 656            # xl_tab row width (1312B, 16B aligned)
    PAIRS = TOTCH // 2
    NS = GPC * 128
    NTILES = NT // 128    # phase-1 iterations
    ADD = mybir.AluOpType.add
    MULT = mybir.AluOpType.mult
    EQ = mybir.AluOpType.is_equal
    AX = mybir.AxisListType.X
    ACT = mybir.ActivationFunctionType
    DR = mybir.MatmulPerfMode.DoubleRow

    nc = bacc.Bacc("TRN2", target_bir_lowering=False, debug=cfg.debug,
                   num_devices=cfg.n_cores)
    xT_d = nc.dram_tensor("xT", [F, NT], BF16, kind="ExternalInput")
    xoT_d = nc.dram_tensor("xoT", [F, NS], BF16, kind="ExternalInput")
    lhst8_d = nc.dram_tensor("lhst8", [PAIRS * 128, 2, 256], FP8, kind="ExternalInput")
    mf_d = nc.dram_tensor("mf", [PAIRS * 128, 256], BF16, kind="ExternalInput")
    srci_d = nc.dram_tensor("srci", [128, TOTCH], I32, kind="ExternalInput")
    bloc_d = nc.dram_tensor("bloc", [128, GPC], F32, kind="ExternalInput")
    wl_d = nc.dram_tensor("wl", [F, HCH], BF16, kind="ExternalInput")
    wr_d = nc.dram_tensor("wr", [F, HCH], BF16, kind="ExternalInput")
    we8_d = nc.dram_tensor("we8", [F, HCH], FP8, kind="ExternalInput")
    attb_d = nc.dram_tensor("attb", [128, HC], BF16, kind="ExternalInput")
    bsumb_d = nc.dram_tensor("bsumb", [128, HCH], F32, kind="ExternalInput")
    biasb_d = nc.dram_tensor("biasb", [128, C], F32, kind="ExternalInput")
    wlin_d = nc.dram_tensor("wlin", [C, 2], F32, kind="ExternalInput")
    cinv_d = nc.dram_tensor("cinv", [G, 1], F32, kind="ExternalInput")
    xl_tab = nc.dram_tensor("xl_tab", [NT, ROWW], BF16, kind="Internal")
    out_d = nc.dram_tensor("out", [G, 2], F32, kind="ExternalOutput")

    NSPL = [(0, 512), (512, HCH)]
    SLAB = 8              # phase-1 xT slab: 8 node-tiles (1024 cols) per DMA

    with tile.TileContext(nc) as tc, ExitStack() as ctx:
        cp = ctx.enter_context(tc.tile_pool(name="const", bufs=1))
        sp = ctx.enter_context(tc.tile_pool(name="small", bufs=6))
        bp = ctx.enter_context(tc.tile_pool(name="big", bufs=8))
        dp = ctx.enter_context(tc.tile_pool(name="dma", bufs=4))
        tp_ = ctx.enter_context(tc.tile_pool(name="tb", bufs=6))

        def cload(name, dram, shape, dt):
            t = cp.tile(shape, dt, tag=name)
            nc.sync.dma_start(t[:], dram.ap())
            return t

        wl = cload("wl", wl_d, [F, HCH], BF16)
        wr = cload("wr", wr_d, [F, HCH], BF16)
        attb = cload("attb", attb_d, [128, HC], BF16)
        bsumb = cload("bsumb", bsumb_d, [128, HCH], F32)
        biasb = cload("biasb", biasb_d, [128, C], F32)
        wlin = cload("wlin", wlin_d, [C, 2], F32)
        cinv = cload("cinv", cinv_d, [G, 1], F32)
        srcs = cload("srcs", srci_d, [128, TOTCH], I32)
        blocs = cload("blocs", bloc_d, [128, GPC], F32)

        ident = cp.tile([128, 128], BF16, tag="ident")
        make_identity(nc, ident[:])
        iotaF = cp.tile([128, 128], F32, tag="iotaF")
        nc.gpsimd.iota(iotaF[:], pattern=[[1, 128]], base=0, channel_multiplier=0,
                       allow_small_or_imprecise_dtypes=True)

        poolacc = cp.tile([C, G], F32, tag="poolacc")
        nc.gpsimd.memset(poolacc[:], 0.0)

        # ping-pong DoubleRow rhs tiles [k, 2, HCH]: k-tile0 = We8 (written
        # once), k-tile1 = per-group Xr8
        rhs8 = [cp.tile([128, 2, HCH], FP8, tag=f"rhs8_{i}", name=f"rhs8_{i}")
                for i in range(2)]
        for i in range(2):
            nc.sync.dma_start(rhs8[i][:, 0, :], we8_d.ap())

        # ---- phase 1: xl_tab[n] = 16 * (x @ Wl') for ALL nodes.  Runs in its
        # own 4-deep PSUM scope so the PE streams continuously (p-state ramp).
        with tc.tile_pool(name="p1ps", bufs=4, space="PSUM") as ppb:
            for i in range(NTILES):
                if i % SLAB == 0:
                    xsl = tp_.tile([128, SLAB * 128], BF16, tag="xsl")
                    w = min(SLAB * 128, NT - i * 128)
                    nc.sync.dma_start(xsl[:, 0:w], xT_d.ap()[:, i * 128:i * 128 + w])
                xin = xsl[:, (i % SLAB) * 128:(i % SLAB + 1) * 128]
                tps = ppb.tile([128, HCH], F32, tag="t1")
                for a, b in NSPL:
                    nc.tensor.matmul(tps[:, a:b], lhsT=xin, rhs=wl[:, a:b],
                                     start=True, stop=True)
                trow = tp_.tile([128, HCH], BF16, tag="trow")
                if i % 2 == 0:
                    nc.scalar.copy(trow[:], tps[:])
                else:
                    nc.vector.tensor_copy(out=trow[:], in_=tps[:])
                q = nc.sync if i % 2 == 0 else nc.gpsimd
                q.dma_start(xl_tab.ap()[i * 128:(i + 1) * 128, 0:HCH], trow[:])

        pp = ctx.enter_context(tc.tile_pool(name="ps", bufs=1, space="PSUM"))
        ppm = ctx.enter_context(tc.tile_pool(name="psm", bufs=2, space="PSUM"))
        ppt = ctx.enter_context(tc.tile_pool(name="pst", bufs=2, space="PSUM"))

        # ---- phase 2: message passing over groups/chunks
        t0 = 0
        for g in range(GPC):
            # group-level: xr = xoT_g.T @ Wr (x16) + bsum -> fp8 slot1
            xgT = sp.tile([128, 128], BF16, tag="xgT")
            nc.sync.dma_start(xgT[:], xoT_d.ap()[:, g * 128:(g + 1) * 128])
            xr_ps = ppm.tile([128, HCH], F32, tag="m")
            for a, b in NSPL:
                nc.tensor.matmul(xr_ps[:, a:b], lhsT=xgT[:], rhs=wr[:, a:b],
                                 start=True, stop=True)
            xr_sb = bp.tile([128, HCH], F32, tag="xr_sb")
            nc.vector.tensor_tensor(out=xr_sb[:], in0=xr_ps[:], in1=bsumb[:], op=ADD)
            rhs_g = rhs8[g % 2]
            nc.scalar.copy(rhs_g[:, 1, :], xr_sb[:])

            scat = pp.tile([128, HCH], F32, tag="scat")
            for k in range(Kg[g]):
                t = t0 + k
                first, last = k == 0, k == Kg[g] - 1
                pi, pj = t // 2, t % 2
                if pj == 0:
                    lh8 = dp.tile([128, 2, 256], FP8, tag="lh8")
                    nc.sync.dma_start(lh8[:], lhst8_d.ap()[pi * 128:(pi + 1) * 128, :, :])
                    mfp = dp.tile([128, 256], BF16, tag="mfp")
                    nc.sync.dma_start(mfp[:], mf_d.ap()[pi * 128:(pi + 1) * 128, :])
                xlr = bp.tile([128, ROWW], BF16, tag="xlr")
                nc.gpsimd.indirect_dma_start(
                    out=xlr[:], out_offset=None, in_=xl_tab.ap(),
                    in_offset=bass.IndirectOffsetOnAxis(ap=srcs[:, t:t + 1], axis=0))

                # m = xl (identity inject, exact bf16) + [eaT; MT].T@[We8; Xr8]
                m_ps = ppm.tile([128, HCH], F32, tag="m")
                for a, b in NSPL:
                    nc.tensor.matmul(m_ps[:, a:b], lhsT=ident[:], rhs=xlr[:, a:b],
                                     start=True, stop=True)
                for a, b in NSPL:
                    nc.tensor.matmul(m_ps[:, a:b], lhsT=lh8[:, :, pj * 128:(pj + 1) * 128],
                                     rhs=rhs_g[:, :, a:b], start=False, stop=True,
                                     perf_mode=DR, skip_group_check=True)

                # attention: relu undoes x16 via activation scale; a2 extracted
                # early so m_ps frees after two ACT reads
                mrelu = bp.tile([128, HC], BF16, tag="mrelu")
                nc.scalar.activation(mrelu[:], m_ps[:, 0:HC], ACT.Relu,
                                     scale=1.0 / WSCALE)
                a2 = sp.tile([128, H], F32, tag="a2")
                nc.scalar.mul(a2[:], m_ps[:, HC:HCH], 1.0 / WSCALE)
                prod = bp.tile([128, HC], BF16, tag="prod")
                nc.vector.tensor_tensor(out=prod[:], in0=mrelu[:], in1=attb[:], op=MULT)
                ar = sp.tile([128, H], F32, tag="ar")
                nc.vector.tensor_reduce(out=ar[:],
                                        in_=prod[:].rearrange("p (h c) -> p h c", h=H),
                                        axis=AX, op=ADD)
                al = sp.tile([128, H], F32, tag="al")
                nc.vector.tensor_tensor(out=al[:], in0=a2[:], in1=ar[:], op=ADD)
                v = bp.tile([128, HCH], BF16, tag="v")
                nc.scalar.activation(v[:, HC:HCH], al[:], ACT.Exp)
                nc.vector.tensor_tensor(out=v[:, 0:HC].rearrange("p (h c) -> p h c", h=H),
                                        in0=xlr[:, 0:HC].rearrange("p (h c) -> p h c", h=H),
                                        in1=v[:, HC:HCH].to_broadcast([128, H, C]),
                                        op=MULT)

                mft = mfp[:, pj * 128:(pj + 1) * 128]
                nc.tensor.matmul(scat[:, 0:512], lhsT=mft,
                                 rhs=v[:, 0:512], start=first, stop=last)
                nc.tensor.matmul(scat[:, 512:HCH], lhsT=mft,
                                 rhs=v[:, 512:HCH], start=first, stop=last)
            t0 += Kg[g]

            # group postprocess: divide by denom (x16 value scale), head-mean,
            # bias, relu, pool
            d10 = sp.tile([128, H], F32, tag="d10")
            nc.vector.tensor_scalar(out=d10[:], in0=scat[:, HC:HCH],
                                    scalar1=1e-12, scalar2=float(H) * WSCALE,
                                    op0=ADD, op1=MULT)
            rec = sp.tile([128, H], F32, tag="rec")
            nc.vector.reciprocal(rec[:], d10[:])
            osc = bp.tile([128, HC], F32, tag="osc")
            nc.vector.tensor_tensor(out=osc[:].rearrange("p (h c) -> p h c", h=H),
                                    in0=scat[:, 0:HC].rearrange("p (h c) -> p h c", h=H),
                                    in1=rec[:].to_broadcast([128, H, C]), op=MULT)
            red = sp.tile([128, C], F32, tag="red")
            nc.vector.tensor_reduce(out=red[:],
                                    in_=osc[:].rearrange("p (h c) -> p c h", h=H),
                                    axis=AX, op=ADD)
            rb = sp.tile([128, C], F32, tag="rb")
            nc.vector.tensor_tensor(out=rb[:], in0=red[:], in1=biasb[:], op=ADD)
            og = sp.tile([128, C], BF16, tag="og")
            nc.scalar.activation(og[:], rb[:], ACT.Relu)
            oh = sp.tile([128, G], BF16, tag="oh")
            nc.vector.tensor_scalar(out=oh[:], in0=iotaF[:, :G],
                                    scalar1=blocs[:, g:g + 1], scalar2=None, op0=EQ)
            pool_ps = ppt.tile([C, G], F32, tag="tp")
            nc.tensor.matmul(pool_ps[:], lhsT=og[:], rhs=oh[:], start=True, stop=True)
            nc.vector.tensor_tensor(out=poolacc[:], in0=pool_ps[:], in1=poolacc[:], op=ADD)

        fin_ps = ppt.tile([G, 2], F32, tag="tp")
        nc.tensor.matmul(fin_ps[:], lhsT=poolacc[:], rhs=wlin[:], start=True, stop=True)
        fin = sp.tile([G, 2], F32, tag="fin")
        nc.vector.tensor_scalar(out=fin[:], in0=fin_ps[:], scalar1=cinv[:, :1],
                                scalar2=None, op0=MULT)
        nc.sync.dma_start(out_d.ap(), fin[:])

    nc.compile()
    return nc


def postprocess(core_outs, b_lin):
    return np.sum(np.stack(core_outs), axis=0).astype(np.float32) + b_lin


# ---------------------------------------------------------------------------
# Self-contained entry point: kernel(**inputs) -> np.ndarray [G, 2]
# ---------------------------------------------------------------------------
_G_GRAPHS = 64
_N_CORES = 8


def kernel(**inputs):
    import numpy as _np
    inp = {k: _np.asarray(v) for k, v in inputs.items()}
    cfg, in_maps, b_lin = preprocess(inp, _N_CORES, _G_GRAPHS)
    nc = build_kernel(cfg)
    from concourse.bass_utils import run_bass_kernel_spmd
    res = run_bass_kernel_spmd(nc, in_maps, list(range(_N_CORES)), trace=False)
    outs = [res.results[c]["out"] for c in range(_N_CORES)]
    return postprocess(outs, b_lin)
